# revision 8
# baseline (speedup 1.0000x reference)
"""AdaptivePruner Trainium2 kernel: gini-routed 1/2-level db4 DWT lowpass.

Strategy
--------
- Routing (gini > SOGLIA) is computed on host with jax-on-CPU, bit-matching
  the reference's float32 arithmetic (one row's gini sits 4e-7 from the
  threshold, so arithmetic-order fidelity matters).
- The DWT is expressed as a single dense matmul per sample: out[b] =
  Cx_b.T @ x[b] where Cx_b (197x102) is the host-selected composite matrix
  (level-1 conv, or level-1 o level-2 conv zero-padded, with an identity
  entry passing the cls token through). Mask is reconstructed on host.
- Pure data parallelism: batch 256 -> 8 NeuronCores x 32 samples.
- On device, per sample: two contiguous input DMAs (K-split 128+69), a
  weight DMA, K-accumulated matmuls into PSUM (bf16, full PE rate), PSUM ->
  SBUF copy, one contiguous output DMA. Memory-bound by design.
"""

import os
import sys

import numpy as np

for _p in ("/opt/trn_rl_repo", "/root/.axon_site/_ro/trn_rl_repo"):
    if os.path.isdir(_p) and _p not in sys.path:
        sys.path.append(_p)

import concourse.bass as bass
import concourse.bacc as bacc
import concourse.mybir as mybir
from concourse.tile import TileContext
from concourse.bass_utils import run_bass_kernel_spmd

SOGLIA = 0.333
DB4_H = np.array([0.23037781330885523, 0.7148465705525415, 0.6308807679295904,
                  -0.02798376941698385, -0.18703481171888114, 0.030841381835986965,
                  0.032883011666982945, -0.010597401784997278], dtype=np.float32)

B, N_TOK, D = 256, 197, 768          # x: (B, 197, 768)
NP_ = 196                            # patch tokens
LEN1, LEN2 = 101, 54                 # dwt output lengths
OUT_TOK = LEN1 + 1                   # 102 = cls + padded patches
N_CORES = 8
B_LOC = B // N_CORES                 # 32
KA, KB = 128, N_TOK - 128            # contraction split 128 + 69
DC = 384                             # free-dim chunk (2 x 384 = 768)

LAST_RESULT = None                   # BassKernelResults of the last run


def _conv_mats():
    """Composite DWT matrices in float64: Cx1/Cx2 (197, 102).

    Row 0 / col 0 pass the cls token through; rows 1+k / cols 1+t hold the
    level-1 (C1) or level-1 compose level-2 (C2, zero-padded to 101 cols)
    lowpass conv-as-matmul weights: y[t] = sum_l h[l] * patch[2t + l - 6].
    """
    h = DB4_H.astype(np.float64)
    C1 = np.zeros((NP_, LEN1))
    for t in range(LEN1):
        for l in range(8):
            k = 2 * t + l - 6
            if 0 <= k < NP_:
                C1[k, t] = h[l]
    M2 = np.zeros((LEN1, LEN2))
    for s in range(LEN2):
        for l in range(8):
            j = 2 * s + l - 6
            if 0 <= j < LEN1:
                M2[j, s] = h[l]
    C2 = np.zeros((NP_, LEN1))
    C2[:, :LEN2] = C1 @ M2
    out = []
    for C in (C1, C2):
        Cx = np.zeros((N_TOK, OUT_TOK))
        Cx[0, 0] = 1.0
        Cx[1:, 1:] = C
        out.append(Cx)
    return out


def _level2_host(cam: np.ndarray) -> np.ndarray:
    """Replicate reference compute_gini bit-exactly with jax on CPU."""
    import jax
    import jax.numpy as jnp

    cpu = jax.devices("cpu")[0]
    with jax.default_device(cpu):
        probs = jnp.asarray(cam)
        n = probs.shape[1]
        sp = jnp.sort(probs, axis=1)
        idx = jnp.arange(1, n + 1, dtype=sp.dtype)
        gini = 2.0 * (idx * sp).sum(axis=1) / (n * sp.sum(axis=1) + 1e-8) - (n + 1) / n
        return np.asarray(gini > SOGLIA)


def _build_nc():
    nc = bacc.Bacc(None, target_bir_lowering=False, debug=False)
    f32 = mybir.dt.float32
    bf16 = mybir.dt.bfloat16
    x = nc.declare_dram_parameter("x", [B_LOC, N_TOK, D], f32, isOutput=False)
    w = nc.declare_dram_parameter("w", [B_LOC, N_TOK, OUT_TOK], bf16, isOutput=False)
    out = nc.declare_dram_parameter("out", [B_LOC, OUT_TOK, D], f32, isOutput=True)

    with TileContext(nc) as tc:
        with (
            tc.tile_pool(name="sb", bufs=4) as pool,
            tc.tile_pool(name="ps", bufs=3, space="PSUM") as pp,
        ):
            for b in range(B_LOC):
                xa = pool.tile([KA, D], f32, tag="xa")
                xb = pool.tile([KB, D], f32, tag="xb")
                wa = pool.tile([KA, OUT_TOK], bf16, tag="wa")
                wb = pool.tile([KB, OUT_TOK], bf16, tag="wb")
                nc.sync.dma_start(xa[:], x[b, 0:KA, :])
                nc.sync.dma_start(xb[:], x[b, KA:N_TOK, :])
                nc.sync.dma_start(wa[:], w[b, 0:KA, :])
                nc.sync.dma_start(wb[:], w[b, KA:N_TOK, :])
                xab = pool.tile([KA, D], bf16, tag="xab")
                xbb = pool.tile([KB, D], bf16, tag="xbb")
                nc.scalar.copy(xab[:], xa[:])
                nc.scalar.copy(xbb[:], xb[:])
                ot = pool.tile([OUT_TOK, D], f32, tag="ot")
                ps = pp.tile([OUT_TOK, D], f32, tag="ps")
                for c0, cn in ((0, 512), (512, 256)):  # PSUM-bank-aligned chunks
                    nc.tensor.matmul(ps[:, c0:c0 + cn], wa[:], xab[:, c0:c0 + cn],
                                     start=True, stop=False)
                    nc.tensor.matmul(ps[:, c0:c0 + cn], wb[:], xbb[:, c0:c0 + cn],
                                     start=False, stop=True)
                nc.vector.tensor_copy(ot[:], ps[:])
                nc.sync.dma_start(out[b, :, :], ot[:])
    nc.compile()
    return nc


def kernel(x: np.ndarray, cls_attention_map: np.ndarray):
    global LAST_RESULT
    import ml_dtypes

    x = np.ascontiguousarray(x, dtype=np.float32)
    cam = np.ascontiguousarray(cls_attention_map, dtype=np.float32)

    level2 = _level2_host(cam)                       # (B,) bool
    Cx1, Cx2 = _conv_mats()
    Wpair = np.stack([Cx1, Cx2]).astype(ml_dtypes.bfloat16)   # (2,197,102)
    w_all = Wpair[level2.astype(np.int64)]           # (B,197,102) bf16

    nc = _build_nc()
    in_maps = [
        {"x": x[i * B_LOC:(i + 1) * B_LOC], "w": w_all[i * B_LOC:(i + 1) * B_LOC]}
        for i in range(N_CORES)
    ]
    LAST_RESULT = run_bass_kernel_spmd(nc, in_maps, core_ids=list(range(N_CORES)))
    final_x = np.concatenate([r["out"] for r in LAST_RESULT.results], axis=0)

    out_len = np.where(level2, LEN2, LEN1)
    mask = np.arange(LEN1)[None, :] < out_len[:, None]
    mask = np.concatenate([np.ones((B, 1), dtype=bool), mask], axis=1)
    return final_x, mask
